# revision 1
# baseline (speedup 1.0000x reference)
"""Bass/Tile TRN2 kernel for an (intentionally quirky) nn.MultiHeadAttention.

Problem shapes: B=8, S=256, D=4096, H=16, HD=256.
Sharding: pure data-parallel - one batch element per NeuronCore (8 cores).

Math (per batch b, with m[j] = float(mask[b, j] != 0)):
    Q = (x_q @ Wq.T + bq) / 16           (1/sqrt(HD) folded into Q)
    K = (x_k @ Wk.T + bk) * m[c mod 256] (the module masks head-dim channels;
                                          masking K only is exact since m^2 = m)
    V = x_v @ Wv.T + bv
    per head h: S_T[t, s] = sum_hd K_h[t,hd] Q_h[s,hd]   (scores, transposed)
                P = exp(S_T)              (no max-sub needed; |scores| <~ 6)
                r[s] = 1 / sum_t P[t, s]  (via ones-matmul over partitions)
                A_h[hd, s] = (sum_t V_h[t,hd] P[t,s]) * r[s]
    out_T = (Wo/64) @ concat_h(A_h) + bo  ->  host transposes back.

All matmuls run in bf16 (1 cycle/row on PE) with fp32 PSUM accumulation.
"""

import os
import sys
import types

sys.path.insert(0, "/opt/trn_rl_repo")

import numpy as np
import ml_dtypes

import concourse.bass as bass
import concourse.mybir as mybir
import concourse.tile as tile
from concourse.vector_clock import ScopedClock

BF16 = mybir.dt.bfloat16
F32 = mybir.dt.float32
NPBF16 = ml_dtypes.bfloat16

B, S, D, H = 8, 256, 4096, 16
HD = D // H          # 256
NK = D // 128        # 32 k-tiles of 128
NC = D // 128        # 32 output-channel tiles of 128
NG = 4               # dout groups of 8 tiles (8 PSUM banks)
N_CORES = 8

_drain_patched = False


def _patch_tile_drain():
    """This container's walrus build accepts only one sync-wait per
    instruction; Tile's exit drain collects one wait per logical processor.
    Split the waits across a chain of drains."""
    global _drain_patched
    if _drain_patched:
        return
    _drain_patched = True

    def patched(self, tick_clock, wait_clock):
        drain_inst = self.nc.sync.drain()
        inst = drain_inst.ins
        wait_clock.add_sem_waits(inst, ScopedClock({None: tick_clock.global_clock}))
        si = inst.sync_info
        if si is not None and len(si.on_wait) > 1:
            waits = list(si.on_wait)
            inst.sync_info = mybir.SyncInfo(
                on_wait=waits[:1], on_update=list(si.on_update)
            )
            for i in range(1, len(waits)):
                extra = self.nc.sync.drain()
                extra.ins.sync_info = mybir.SyncInfo(
                    on_wait=waits[i : i + 1], on_update=[]
                )
        self.nc.all_engine_barrier()
        popped = self.nc._tile_sem_poison_stack.pop()
        assert popped is self._sem_poison
        self.nc.clear_and_free_semaphores(list(self.sems.allocated().values()))
        self.nc.all_engine_barrier()

    tile.TileContext._drain_and_barrier = patched


_ldw_patched = False


def _patch_ldw_opt():
    """Optionally flip walrus --enable-ldw-opt (BASS_LDW_OPT=1) so LDWEIGHTS
    overlaps matmuls via the background weight buffer."""
    global _ldw_patched
    if _ldw_patched or os.environ.get("BASS_LDW_OPT") != "1":
        return
    _ldw_patched = True
    import concourse.bass_utils as bu

    orig_run = bu.run_command

    def wrapped(argv, **kwargs):
        argv = [
            a.replace("--enable-ldw-opt=false", "--enable-ldw-opt=true")
            if isinstance(a, str)
            else a
            for a in argv
        ]
        return orig_run(argv, **kwargs)

    bu.run_command = wrapped


_bir_patched = False


def _patch_bir_wait_split():
    """This walrus build accepts a single sync-wait per instruction. Tile's
    wait-assignment emits up to 4. Split them in the serialized BIR: extra
    waits are carried by no-op RegisterMove instructions (imm 0 ->
    {Engine}_zero) inserted just before the overloaded instruction on the
    same engine. Monotonic sem-ge waits make sequential waiting equivalent
    to simultaneous waiting."""
    global _bir_patched
    if _bir_patched:
        return
    _bir_patched = True
    import json as _json

    import concourse.bass2jax as b2j

    orig_compile = b2j.compile_bir_kernel

    def split_waits(bir_str):
        m = _json.loads(bir_str)
        changed = False
        for fn in m.get("functions", []):
            for blk in fn.get("blocks", []):
                insts = blk.get("instructions", [])
                out = []
                for inst in insts:
                    si = inst.get("sync_info") or {}
                    waits = si.get("on_wait") or []
                    if len(waits) > 1 and all(
                        w.get("wait_mode") == "sem-ge-imm" for w in waits
                    ):
                        changed = True
                        eng = inst["engine"]
                        for i, w in enumerate(waits[:-1]):
                            out.append(
                                {
                                    "debug": inst.get("debug", 0),
                                    "engine": eng,
                                    "ins": [
                                        {
                                            "dtype": "int32",
                                            "kind": "imm_value",
                                            "value": 0,
                                        }
                                    ],
                                    "name": f"{inst['name']}_w{i}",
                                    "opcode": "RegisterMove",
                                    "outs": [
                                        {
                                            "dtype": "int32",
                                            "kind": "register_access",
                                            "regref": f"{eng}_zero",
                                        }
                                    ],
                                    "sync_info": {
                                        "on_update": [],
                                        "on_wait": [w],
                                    },
                                }
                            )
                        inst["sync_info"] = {
                            "on_update": si.get("on_update") or [],
                            "on_wait": [waits[-1]],
                        }
                    out.append(inst)
                blk["instructions"] = out
        if not changed:
            return bir_str
        return _json.dumps(m).encode()

    def wrapped(ant_bir_str, *args, **kwargs):
        return orig_compile(split_waits(ant_bir_str), *args, **kwargs)

    b2j.compile_bir_kernel = wrapped


def _install_ntff_hook():
    """Recreate the missing antenv.axon_hooks glue so trace=True can profile."""
    if "antenv.axon_hooks" in sys.modules:
        return
    mod = types.ModuleType("antenv.axon_hooks")
    mod._hook = None
    mod.set_axon_ntff_profile_hook = lambda h: setattr(mod, "_hook", h)
    mod.get_axon_ntff_profile_hook = lambda: mod._hook
    sys.modules["antenv.axon_hooks"] = mod
    try:
        import antenv

        antenv.axon_hooks = mod
        if "/root/.axon_site" not in sys.path:
            sys.path.insert(0, "/root/.axon_site")
        from trn_agent_boot.trn_boot import _ntff_profile_via_ctypes

        mod._hook = _ntff_profile_via_ctypes("/opt/axon/libaxon_pjrt.so")
        import concourse.bass_utils as bu

        bu.upload_artifacts = lambda tmpdir: tmpdir
    except Exception:
        pass


def build_nc():
    _patch_tile_drain()
    nc = bass.Bass()

    xq = nc.dram_tensor("xq", [128, NK, S], BF16, kind="ExternalInput")
    xk = nc.dram_tensor("xk", [128, NK, S], BF16, kind="ExternalInput")
    xv = nc.dram_tensor("xv", [128, NK, S], BF16, kind="ExternalInput")
    # weight slabs: [group of 2048 dout cols][k-tile][128 k][2048]
    wq = nc.dram_tensor("wq", [4, NK // 2, 128, 2048], BF16, kind="ExternalInput")
    wk = nc.dram_tensor("wk", [4, NK // 2, 128, 2048], BF16, kind="ExternalInput")
    wo = nc.dram_tensor("wo", [4, NK // 2, 128, 2048], BF16, kind="ExternalInput")
    wv = nc.dram_tensor("wv", [2, NK, 128, 2048], BF16, kind="ExternalInput")
    bqv = nc.dram_tensor("bqv", [128, NC], F32, kind="ExternalInput")
    bkv = nc.dram_tensor("bkv", [128, NC], F32, kind="ExternalInput")
    kmv = nc.dram_tensor("kmv", [128, NC], F32, kind="ExternalInput")
    bov = nc.dram_tensor("bov", [128, NC], F32, kind="ExternalInput")
    bvv = nc.dram_tensor("bvv", [1, D], BF16, kind="ExternalInput")
    out = nc.dram_tensor("out", [NC, 128, S], F32, kind="ExternalOutput")

    Ident = mybir.ActivationFunctionType.Identity
    Exp = mybir.ActivationFunctionType.Exp

    with tile.TileContext(nc) as tc:
        from contextlib import ExitStack

        with ExitStack() as ctx:
            resid = ctx.enter_context(tc.tile_pool(name="resid", bufs=1))
            wpool = ctx.enter_context(tc.tile_pool(name="wch", bufs=8))
            outp = ctx.enter_context(tc.tile_pool(name="outp", bufs=2))
            drp = ctx.enter_context(tc.tile_pool(name="drp", bufs=2, space="DRAM"))

            # ---- resident SBUF tensors ----
            qt_sb = resid.tile([128, NC, S], BF16, tag="qt")  # Q^T  [dout, s]
            kt_sb = resid.tile([128, NC, S], BF16, tag="kt")  # K^T  [dout, t]
            v0_sb = resid.tile([128, D], BF16, tag="v0")      # V[t=0:128, c]
            v1_sb = resid.tile([128, D], BF16, tag="v1")      # V[t=128:256, c]
            at_sb = resid.tile([128, NC, S], BF16, tag="at")  # attn^T [c, s]
            xv_sb = resid.tile([128, NK, S], BF16, tag="xv")
            ball = resid.tile([128, 4, NC], F32, tag="ball")  # bq,bk,km,bo
            bv_sb = resid.tile([1, D], BF16, tag="bv")
            ones_a = resid.tile([128, 128], BF16, tag="ones")
            ones1 = ones_a[0:1, :]
            ones128 = ones_a[:, 0:1]
            bq_sb = ball[:, 0, :]
            bk_sb = ball[:, 1, :]
            km_sb = ball[:, 2, :]
            bo_sb = ball[:, 3, :]

            nc.vector.memset(ones_a[:], 1.0)

            # ---- transposed-output projection (Q, K, and out_linear) ----
            # 16 dout tiles per group; two tiles share one [128,512] PSUM bank
            def proj_t(w_dram, x_sb, dst_sb, bias_sb, scale_sb, pspool, store=None):
                for g in range(4):
                    ps = [
                        pspool.tile([128, S], F32, name="ps", tag="ps")
                        for _ in range(8)
                    ]
                    for kp in range(NK // 2):
                        ch = wpool.tile([128, 2048], BF16, name="ch", tag="ch")
                        eng = nc.sync if kp % 2 == 0 else nc.gpsimd
                        eng.dma_start(out=ch[:], in_=w_dram[g, kp])
                        for half in range(2):
                            kt = 2 * kp + half
                            for j in range(8):
                                nc.tensor.matmul(
                                    ps[j][:],
                                    lhsT=ch[
                                        :, half * 1024 + j * 128 : half * 1024 + (j + 1) * 128
                                    ],
                                    rhs=x_sb[:, kt, :],
                                    start=(kt == 0),
                                    stop=(kt == NK - 1),
                                )
                    for j in range(8):
                        c = g * 8 + j
                        src = ps[j][:]
                        scale = scale_sb[:, c : c + 1] if scale_sb is not None else 1.0
                        if store is None:
                            nc.scalar.activation(
                                out=dst_sb[:, c, :],
                                in_=src,
                                func=Ident,
                                bias=bias_sb[:, c : c + 1],
                                scale=scale,
                            )
                        else:
                            ot = outp.tile([128, S], F32, name="ot", tag="ot")
                            nc.scalar.activation(
                                out=ot[:],
                                in_=src,
                                func=Ident,
                                bias=bias_sb[:, c : c + 1],
                                scale=scale,
                            )
                            nc.sync.dma_start(out=store[c], in_=ot[:])

            with tc.tile_pool(name="xqk", bufs=1) as xqkp:
                xq_sb = xqkp.tile([128, NK, S], BF16, tag="xq")
                nc.sync.dma_start(out=xq_sb[:], in_=xq[:])
                xk_sb = xqkp.tile([128, NK, S], BF16, tag="xk")
                nc.gpsimd.dma_start(out=xk_sb[:], in_=xk[:])
                nc.sync.dma_start(out=xv_sb[:], in_=xv[:])
                nc.gpsimd.dma_start(out=ball[:, 0, :], in_=bqv[:])
                nc.gpsimd.dma_start(out=ball[:, 1, :], in_=bkv[:])
                nc.gpsimd.dma_start(out=ball[:, 2, :], in_=kmv[:])
                nc.gpsimd.dma_start(out=ball[:, 3, :], in_=bov[:])
                nc.gpsimd.dma_start(out=bv_sb[:], in_=bvv[:])
                with tc.tile_pool(name="psqk", bufs=8, space="PSUM") as psqk:
                    proj_t(wq, xq_sb, qt_sb, bq_sb, None, psqk)
                    proj_t(wk, xk_sb, kt_sb, bk_sb, km_sb, psqk)

            # ---- V projection (x stationary, natural layout [t, c]) ----
            # per round r: 2048 dout cols; 8 banks = 2 t-halves x 4 chunks
            if True:
                with tc.tile_pool(name="psv", bufs=8, space="PSUM") as psv:
                    for r in range(2):
                        pv = [
                            [
                                psv.tile([128, 512], F32, name="pv", tag="pv")
                                for _ in range(4)
                            ]
                            for _ in range(2)
                        ]
                        for tt in range(2):
                            for di in range(4):
                                sl = slice(r * 2048 + di * 512, r * 2048 + di * 512 + 512)
                                nc.tensor.matmul(
                                    pv[tt][di][:], lhsT=ones1[:], rhs=bv_sb[:, sl],
                                    start=True, stop=False,
                                )
                        for kt in range(NK):
                            ch = wpool.tile([128, 2048], BF16, name="ch", tag="ch")
                            eng = nc.sync if kt % 2 == 0 else nc.gpsimd
                            eng.dma_start(out=ch[:], in_=wv[r, kt])
                            last = kt == NK - 1
                            for tt in range(2):
                                for di in range(4):
                                    nc.tensor.matmul(
                                        pv[tt][di][:],
                                        lhsT=xv_sb[:, kt, tt * 128 : (tt + 1) * 128],
                                        rhs=ch[:, di * 512 : (di + 1) * 512],
                                        start=False,
                                        stop=last,
                                    )
                        for tt in range(2):
                            vdst = v0_sb if tt == 0 else v1_sb
                            for di in range(4):
                                sl = slice(
                                    r * 2048 + di * 512, r * 2048 + di * 512 + 512
                                )
                                nc.scalar.activation(
                                    out=vdst[:, sl], in_=pv[tt][di][:], func=Ident
                                )

            # ---- attention ----
            # phase A: per head scores^T, exp, col sums; bounce recips in halves
            with tc.tile_pool(name="attb", bufs=1) as attb:
                et4 = [
                    attb.tile([128, 8, S], BF16, name="et", tag=f"et{i}")
                    for i in range(4)
                ]
                rinv = [
                    attb.tile([1, 8, S], F32, name="ri", tag=f"ri{i}")
                    for i in range(2)
                ]
                rbc = [
                    attb.tile([128, 8, S], F32, name="rbc", tag=f"rbc{i}")
                    for i in range(2)
                ]
                with (
                    tc.tile_pool(name="psa", bufs=6, space="PSUM") as psa,
                    tc.tile_pool(name="psr", bufs=2, space="PSUM") as psr,
                ):
                    for h in range(H):
                        c0 = 2 * h
                        eth = et4[h // 4][:, (h % 4) * 2 : (h % 4) * 2 + 2, :]
                        for tt in range(2):
                            pss = psa.tile([128, S], F32, name="pa", tag="pa")
                            tsl = slice(tt * 128, (tt + 1) * 128)
                            for j in range(2):
                                nc.tensor.matmul(
                                    pss[:],
                                    lhsT=kt_sb[:, c0 + j, tsl],
                                    rhs=qt_sb[:, c0 + j, :],
                                    start=(j == 0),
                                    stop=(j == 1),
                                )
                            nc.scalar.activation(
                                out=eth[:, tt, :], in_=pss[:], func=Exp
                            )
                        # column sums of exp (over t = partitions) via matmul
                        prs = psr.tile([1, S], F32, name="pr", tag="pr")
                        nc.tensor.matmul(
                            prs[:], lhsT=ones128[:], rhs=eth[:, 0, :],
                            start=True, stop=False,
                        )
                        nc.tensor.matmul(
                            prs[:], lhsT=ones128[:], rhs=eth[:, 1, :],
                            start=False, stop=True,
                        )
                        nc.vector.reciprocal(rinv[h // 8][:, h % 8, :], prs[:])
                        if h % 8 == 7:
                            # bounce this half through DRAM to broadcast
                            # across partitions (only DRAM DMA may replicate)
                            i = h // 8
                            rdr = drp.tile([1, 8, S], F32, name="rdr", tag="rdr")
                            nc.gpsimd.dma_start(out=rdr[:], in_=rinv[i][:])
                            rdr_b = bass.AP(
                                tensor=rdr.tensor,
                                offset=rdr.offset,
                                ap=[[0, 128], [S, 8], [1, S]],
                            )
                            nc.gpsimd.dma_start(out=rbc[i][:], in_=rdr_b)

                # phase B: attention @ V, normalized on PSUM->SBUF copy
                with tc.tile_pool(name="psc", bufs=6, space="PSUM") as psc:
                    for h in range(H):
                        c0 = 2 * h
                        for j in range(2):
                            csl = slice(h * HD + j * 128, h * HD + (j + 1) * 128)
                            pu = psc.tile([128, S], F32, name="pc", tag="pc")
                            nc.tensor.matmul(
                                pu[:], lhsT=v0_sb[:, csl], rhs=et4[h // 4][:, (h % 4) * 2, :],
                                start=True, stop=False,
                            )
                            nc.tensor.matmul(
                                pu[:], lhsT=v1_sb[:, csl], rhs=et4[h // 4][:, (h % 4) * 2 + 1, :],
                                start=False, stop=True,
                            )
                            nc.vector.tensor_mul(
                                at_sb[:, c0 + j, :], pu[:], rbc[h // 8][:, h % 8, :]
                            )

            # ---- output projection (same transposed structure as Q/K) ----
            with tc.tile_pool(name="pso", bufs=8, space="PSUM") as pso:
                proj_t(wo, at_sb, None, bo_sb, None, pso, store=out)

    return nc


_cached = {}


def _get_nc():
    if "nc" not in _cached:
        _cached["nc"] = build_nc()
    return _cached["nc"]


def _prep_shared(Wq, bq, Wk, bk, Wv, bv, Wo, bo):
    """Host-side weight reorganization (shared across cores)."""
    def chunks2048(W, scale):
        # W.T with dout split into 2 groups of 2048: [2, 32, 128, 2048]
        wt = (W.T * scale).astype(NPBF16)  # [k, dout]
        return np.ascontiguousarray(
            wt.reshape(NK, 128, 2, 2048).transpose(2, 0, 1, 3)
        )

    def chunks_paired(W, scale):
        # [4 groups, 16 kt-pairs, 128, 2048]; slab free dim = [half][1024 dout]
        wt = (W.T * scale).astype(NPBF16)
        return np.ascontiguousarray(
            wt.reshape(16, 2, 128, 4, 1024).transpose(3, 0, 2, 1, 4).reshape(
                4, 16, 128, 2048
            )
        )

    wq_c = chunks_paired(Wq, 1.0 / 16.0)
    wk_c = chunks_paired(Wk, 1.0)
    wo_c = chunks_paired(Wo, 1.0 / 64.0)
    wv_c = chunks2048(Wv, 1.0)

    bqv = np.ascontiguousarray((bq / 16.0).astype(np.float32).reshape(NC, 128).T)
    bov = np.ascontiguousarray(bo.astype(np.float32).reshape(NC, 128).T)
    bvv = np.ascontiguousarray(bv.astype(NPBF16).reshape(1, D))
    return wq_c, wk_c, wv_c, wo_c, bqv, bov, bvv


def build_in_maps(q, k, v, mask, Wq, bq, Wk, bk, Wv, bv, Wo, bo):
    q = np.asarray(q, dtype=np.float32)
    k = np.asarray(k, dtype=np.float32)
    v = np.asarray(v, dtype=np.float32)
    mask = np.asarray(mask)
    Wq, bq = np.asarray(Wq, np.float32), np.asarray(bq, np.float32)
    Wk, bk = np.asarray(Wk, np.float32), np.asarray(bk, np.float32)
    Wv, bv = np.asarray(Wv, np.float32), np.asarray(bv, np.float32)
    Wo, bo = np.asarray(Wo, np.float32), np.asarray(bo, np.float32)

    wq_c, wk_c, wv_c, wo_c, bqv, bov, bvv = _prep_shared(
        Wq, bq, Wk, bk, Wv, bv, Wo, bo
    )

    in_maps = []
    for b in range(B):
        m = (mask[b] != 0).astype(np.float32)  # [256]
        mfull = np.tile(m, H)                  # [4096] mask per channel
        bkv = np.ascontiguousarray((bk * mfull).reshape(NC, 128).T.astype(np.float32))
        kmv = np.ascontiguousarray(mfull.reshape(NC, 128).T.astype(np.float32))

        def xt(x):
            # [128 partition, NK k-tile, S] with 16KB contiguous per partition
            t = x[b].T.reshape(NK, 128, S).swapaxes(0, 1)
            return np.ascontiguousarray(t).astype(NPBF16)

        in_maps.append(
            dict(
                xq=xt(q), xk=xt(k), xv=xt(v),
                wq=wq_c, wk=wk_c, wv=wv_c, wo=wo_c,
                bqv=bqv, bkv=bkv, kmv=kmv, bov=bov, bvv=bvv,
            )
        )
    return in_maps


def unshard(results):
    outs = []
    for b in range(B):
        ot = results[b]["out"]  # [32, 128, 256]
        outs.append(ot.reshape(D, S).T)  # [256, 4096]
    return np.ascontiguousarray(np.stack(outs)).astype(np.float32)


def kernel(q, k, v, mask, Wq, bq, Wk, bk, Wv, bv, Wo, bo):
    _install_ntff_hook()
    _patch_bir_wait_split()
    _patch_ldw_opt()
    nc = _get_nc()
    in_maps = build_in_maps(q, k, v, mask, Wq, bq, Wk, bk, Wv, bv, Wo, bo)

    from concourse.bass_utils import run_bass_kernel_spmd

    res = run_bass_kernel_spmd(nc, in_maps, list(range(N_CORES)))
    return unshard(res.results)



# revision 11
# speedup vs baseline: 1.2092x; 1.2092x over previous
"""Bass/Tile TRN2 kernel for an (intentionally quirky) nn.MultiHeadAttention.

Problem shapes: B=8, S=256, D=4096, H=16, HD=256.
Sharding: pure data-parallel - one batch element per NeuronCore (8 cores).

Math (per batch b, with m[j] = float(mask[b, j] != 0)):
    Q = (x_q @ Wq.T + bq) / 16           (1/sqrt(HD) folded into Q)
    K = x_k @ Wk.T + bk  on surviving channels only (channel c of head h is
        kept iff m[c]; both Q and K are masked in the module, and since
        m^2 = m the masked channels contribute nothing to the scores, so we
        pack only surviving channels: NCH per head, zero-padded)
    V = x_v @ Wv.T + bv
    per head h: S_T[t, s] = sum_ch K_h[t,ch] Q_h[s,ch]   (scores, transposed)
                P = exp(S_T)              (no max-sub needed; |scores| <~ 6)
                r[s] = 1 / sum_t P[t, s]  (ones-matmul + fast reciprocal)
                A_h[hd, s] = (sum_t V_h[t,hd] P[t,s]) * r[s]
    out_T = (Wo/64) @ concat_h(A_h) + bo  ->  host transposes back.

All matmuls run in bf16 (1 cycle/row on PE) with fp32 PSUM accumulation.
Structure tuned from perfetto traces:
  - projections loop dout-tile-inner over all 32 k-tiles (1 PSUM bank per
    accumulator, deep double buffering, 1MB weight-slab DMAs)
  - weight streaming over three DMA queues (sync + scalar + gpsimd)
  - V projection braided between attention mini-phases to hide the
    softmax serialization
  - softmax normalization via reciprocal_approx_fast + PE ones-broadcast
    (no DRAM bounce)
"""

import os
import sys
import types

sys.path.insert(0, "/opt/trn_rl_repo")

import numpy as np
import ml_dtypes

import concourse.bass as bass
import concourse.mybir as mybir
import concourse.tile as tile
from concourse.vector_clock import ScopedClock

BF16 = mybir.dt.bfloat16
F32 = mybir.dt.float32
NPBF16 = ml_dtypes.bfloat16

B, S, D, H = 8, 256, 4096, 16
HD = D // H          # 256
NK = D // 128        # 32 k-tiles of 128
NC = D // 128        # 32 output-channel tiles of 128
N_CORES = 8

_drain_patched = False


def _patch_tile_drain():
    """This container's walrus build accepts only one sync-wait per
    instruction; Tile's exit drain collects one wait per logical processor.
    Split the waits across a chain of drains."""
    global _drain_patched
    if _drain_patched:
        return
    _drain_patched = True

    def patched(self, tick_clock, wait_clock):
        drain_inst = self.nc.sync.drain()
        inst = drain_inst.ins
        wait_clock.add_sem_waits(inst, ScopedClock({None: tick_clock.global_clock}))
        si = inst.sync_info
        if si is not None and len(si.on_wait) > 1:
            waits = list(si.on_wait)
            inst.sync_info = mybir.SyncInfo(
                on_wait=waits[:1], on_update=list(si.on_update)
            )
            for i in range(1, len(waits)):
                extra = self.nc.sync.drain()
                extra.ins.sync_info = mybir.SyncInfo(
                    on_wait=waits[i : i + 1], on_update=[]
                )
        self.nc.all_engine_barrier()
        popped = self.nc._tile_sem_poison_stack.pop()
        assert popped is self._sem_poison
        self.nc.clear_and_free_semaphores(list(self.sems.allocated().values()))
        self.nc.all_engine_barrier()

    tile.TileContext._drain_and_barrier = patched


_bir_patched = False


def _patch_bir_wait_split():
    """This walrus build accepts a single sync-wait per instruction. Tile's
    wait-assignment emits up to 4. Split them in the serialized BIR: extra
    waits are carried by no-op RegisterMove instructions (imm 0 ->
    {Engine}_zero) inserted just before the overloaded instruction on the
    same engine. Monotonic sem-ge waits make sequential waiting equivalent
    to simultaneous waiting."""
    global _bir_patched
    if _bir_patched:
        return
    _bir_patched = True
    import json as _json

    import concourse.bass2jax as b2j
    import concourse.bass_utils as _bu

    orig_compile = b2j.compile_bir_kernel
    orig_compile_bu = _bu.compile_bir_kernel

    def split_waits(bir_str):
        m = _json.loads(bir_str)
        changed = False
        for fn in m.get("functions", []):
            for blk in fn.get("blocks", []):
                insts = blk.get("instructions", [])
                out = []
                for inst in insts:
                    si = inst.get("sync_info") or {}
                    waits = si.get("on_wait") or []
                    if len(waits) > 1 and all(
                        w.get("wait_mode") == "sem-ge-imm" for w in waits
                    ):
                        changed = True
                        eng = inst["engine"]
                        for i, w in enumerate(waits[:-1]):
                            out.append(
                                {
                                    "debug": inst.get("debug", 0),
                                    "engine": eng,
                                    "ins": [
                                        {
                                            "dtype": "int32",
                                            "kind": "imm_value",
                                            "value": 0,
                                        }
                                    ],
                                    "name": f"{inst['name']}_w{i}",
                                    "opcode": "RegisterMove",
                                    "outs": [
                                        {
                                            "dtype": "int32",
                                            "kind": "register_access",
                                            "regref": f"{eng}_zero",
                                        }
                                    ],
                                    "sync_info": {
                                        "on_update": [],
                                        "on_wait": [w],
                                    },
                                }
                            )
                        inst["sync_info"] = {
                            "on_update": si.get("on_update") or [],
                            "on_wait": [waits[-1]],
                        }
                    out.append(inst)
                blk["instructions"] = out
        if not changed:
            return bir_str
        return _json.dumps(m).encode()

    def wrapped(ant_bir_str, *args, **kwargs):
        return orig_compile(split_waits(ant_bir_str), *args, **kwargs)

    def wrapped_bu(ant_bir_str, *args, **kwargs):
        return orig_compile_bu(split_waits(ant_bir_str), *args, **kwargs)

    b2j.compile_bir_kernel = wrapped
    _bu.compile_bir_kernel = wrapped_bu


def _install_ntff_hook():
    """Recreate the missing antenv.axon_hooks glue so trace=True can profile."""
    if "antenv.axon_hooks" in sys.modules:
        return
    mod = types.ModuleType("antenv.axon_hooks")
    mod._hook = None
    mod.set_axon_ntff_profile_hook = lambda h: setattr(mod, "_hook", h)
    mod.get_axon_ntff_profile_hook = lambda: mod._hook
    sys.modules["antenv.axon_hooks"] = mod
    try:
        import antenv

        antenv.axon_hooks = mod
        if "/root/.axon_site" not in sys.path:
            sys.path.insert(0, "/root/.axon_site")
        from trn_agent_boot.trn_boot import _ntff_profile_via_ctypes

        mod._hook = _ntff_profile_via_ctypes("/opt/axon/libaxon_pjrt.so")
        import concourse.bass_utils as bu

        bu.upload_artifacts = lambda tmpdir: tmpdir
    except Exception:
        pass


def _head_slices(h, nch):
    """Partition-range slices covering packed channels [h*nch, (h+1)*nch)
    of a [128, n_tiles, ...] layout; each slice is legal as a matmul
    contraction range (base 0 for >64 rows, base in {0,64} for 64 rows)."""
    out = []
    c = h * nch
    end = (h + 1) * nch
    while c < end:
        t, o = divmod(c, 128)
        n = min(128 - o, end - c)
        out.append((t, o, o + n))
        c += n
    for (_, o, e) in out:
        n = e - o
        assert (n > 64 and o == 0) or (n <= 64 and o in (0, 32, 64, 96))
    return out


def build_nc(nch, use_recip_fast=True, use_f32_bcast=True, use_scalar_dma=True):
    _patch_tile_drain()
    nc = bass.Bass()

    nqt = nch * H // 128  # packed Q/K dout tiles (24 for nch=192)

    xq = nc.dram_tensor("xq", [128, NK, S], BF16, kind="ExternalInput")
    xk = nc.dram_tensor("xk", [128, NK, S], BF16, kind="ExternalInput")
    xv = nc.dram_tensor("xv", [128, NK, S], BF16, kind="ExternalInput")
    # Q/K/O weight slabs: [dout tile][128 k-in-tile][32 kt][128 dout]
    wq = nc.dram_tensor("wq", [nqt, 128, NK, 128], BF16, kind="ExternalInput")
    wk = nc.dram_tensor("wk", [nqt, 128, NK, 128], BF16, kind="ExternalInput")
    wo = nc.dram_tensor("wo", [NC, 128, NK, 128], BF16, kind="ExternalInput")
    # V slabs: [round][kt-pair][128 k][2 kt x 1024 dout]
    wv = nc.dram_tensor("wv", [4, NK // 2, 128, 2048], BF16, kind="ExternalInput")
    bqv = nc.dram_tensor("bqv", [128, nqt], F32, kind="ExternalInput")
    bkv = nc.dram_tensor("bkv", [128, nqt], F32, kind="ExternalInput")
    bov = nc.dram_tensor("bov", [128, NC], F32, kind="ExternalInput")
    bvv = nc.dram_tensor("bvv", [1, D], BF16, kind="ExternalInput")
    out = nc.dram_tensor("out", [NC, 128, S], F32, kind="ExternalOutput")

    Ident = mybir.ActivationFunctionType.Identity
    Exp = mybir.ActivationFunctionType.Exp
    Recip = mybir.ActivationFunctionType.Reciprocal

    with tile.TileContext(nc) as tc:
        from contextlib import ExitStack

        with ExitStack() as ctx:
            resid = ctx.enter_context(tc.tile_pool(name="resid", bufs=1))
            wpool = ctx.enter_context(tc.tile_pool(name="wch", bufs=3))
            vwp = ctx.enter_context(tc.tile_pool(name="vch", bufs=4))
            outp = ctx.enter_context(tc.tile_pool(name="outp", bufs=2))
            pprj = ctx.enter_context(tc.tile_pool(name="pprj", bufs=2, space="PSUM"))
            pvps = ctx.enter_context(tc.tile_pool(name="pvps", bufs=4, space="PSUM"))
            paps = ctx.enter_context(tc.tile_pool(name="paps", bufs=2, space="PSUM"))

            # ---- resident SBUF tensors ----
            qt_sb = resid.tile([128, nqt, S], BF16, tag="qt")  # Q^T packed
            kt_sb = resid.tile([128, nqt, S], BF16, tag="kt")  # K^T packed
            v0_sb = resid.tile([128, D], BF16, tag="v0")       # V[t=0:128, c]
            v1_sb = resid.tile([128, D], BF16, tag="v1")       # V[t=128:256, c]
            at_sb = resid.tile([128, NC, S], BF16, tag="at")   # attn^T [c, s]
            xv_sb = resid.tile([128, NK, S], BF16, tag="xv")
            et_sb = resid.tile([128, H, 2, S], BF16, tag="et")  # exp(scores^T)
            rbc = resid.tile([128, H, S], F32, tag="rbc")       # 1/sum bcast
            rinv = resid.tile([1, H, S], F32, tag="rinv")
            ball = resid.tile([128, 4, NC], F32, tag="ball")
            bv_sb = resid.tile([1, D], BF16, tag="bv")
            ones_a = resid.tile([128, 128], BF16, tag="ones")
            ones1 = ones_a[0:1, :]    # [1, 128] row of ones (V bias lhsT)
            ones128 = ones_a[:, 0:1]  # [128, 1] col of ones (colsum lhsT)
            ones_f = resid.tile([1, 128], F32, tag="onesf")  # f32 bcast lhsT
            bq_sb = ball[:, 0, 0:nqt]
            bk_sb = ball[:, 1, 0:nqt]
            bo_sb = ball[:, 2, :]

            nc.vector.memset(ones_a[:], 1.0)
            nc.vector.memset(ones_f[:], 1.0)

            qs = [nc.sync, nc.scalar, nc.gpsimd] if use_scalar_dma else [
                nc.sync, nc.gpsimd]
            qi = [0]

            def rrq():
                e = qs[qi[0] % len(qs)]
                qi[0] += 1
                return e

            # ---- startup DMAs (need-order; xq first, then wq slabs) ----
            xq_pool = ctx.enter_context(tc.tile_pool(name="xq", bufs=1))
            xq_sb = xq_pool.tile([128, NK, S], BF16, tag="xq")
            xk_sb = xq_pool.tile([128, NK, S], BF16, tag="xk")
            for c in range(4):
                rrq().dma_start(
                    out=xq_sb[:, c * 8 : (c + 1) * 8, :],
                    in_=xq[:, c * 8 : (c + 1) * 8, :],
                )
            nc.gpsimd.dma_start(out=ball[:, 0, 0:nqt], in_=bqv[:])
            nc.gpsimd.dma_start(out=ball[:, 1, 0:nqt], in_=bkv[:])
            nc.gpsimd.dma_start(out=ball[:, 2, :], in_=bov[:])
            nc.gpsimd.dma_start(out=bv_sb[:], in_=bvv[:])

            # ---- transposed-output projection, dout-tile-inner over k ----
            # prefetch: {dt: [callable]} -> extra DMAs issued at that tile
            def proj_t(w_dram, x_sb, n_dt, dst_sb, bias_sb, store=None,
                       prefetch=None):
                for dt in range(n_dt):
                    wch = wpool.tile([128, NK, 128], BF16, name="wch", tag="wch")
                    rrq().dma_start(out=wch[:], in_=w_dram[dt])
                    if prefetch and dt in prefetch:
                        for fn in prefetch[dt]:
                            fn()
                    ps = pprj.tile([128, 512], F32, name="ps", tag="ps")
                    acc = ps[:, 0:256]
                    for kt in range(NK):
                        nc.tensor.matmul(
                            acc,
                            lhsT=wch[:, kt, :],
                            rhs=x_sb[:, kt, :],
                            start=(kt == 0),
                            stop=(kt == NK - 1),
                        )
                    if store is None:
                        nc.scalar.activation(
                            out=dst_sb[:, dt, :],
                            in_=acc,
                            func=Ident,
                            bias=bias_sb[:, dt : dt + 1],
                        )
                    else:
                        ot = outp.tile([128, S], F32, name="ot", tag="ot")
                        nc.scalar.activation(
                            out=ot[:],
                            in_=acc,
                            func=Ident,
                            bias=bias_sb[:, dt : dt + 1],
                        )
                        (nc.sync if dt % 2 == 0 else nc.gpsimd).dma_start(
                            out=store[dt], in_=ot[:]
                        )

            def xload(dst, src, c):
                def fn():
                    rrq().dma_start(
                        out=dst[:, c * 8 : (c + 1) * 8, :],
                        in_=src[:, c * 8 : (c + 1) * 8, :],
                    )
                return fn

            # V weight chunks: allocate tile + DMA (possibly prefetched early)
            def vch_load(r, kp):
                vch = vwp.tile([128, 2048], BF16, name="vch", tag="vch")
                rrq().dma_start(out=vch[:], in_=wv[r, kp])
                return vch

            proj_t(wq, xq_sb, nqt, qt_sb, bq_sb,
                   prefetch={12: [xload(xk_sb, xk, 0), xload(xk_sb, xk, 1)],
                             16: [xload(xk_sb, xk, 2), xload(xk_sb, xk, 3)]})

            warm_v0 = []
            proj_t(wk, xk_sb, nqt, kt_sb, bk_sb,
                   prefetch={10: [xload(xv_sb, xv, 0), xload(xv_sb, xv, 1)],
                             14: [xload(xv_sb, xv, 2), xload(xv_sb, xv, 3)],
                             20: [lambda: warm_v0.append(vch_load(0, 0))],
                             22: [lambda: warm_v0.append(vch_load(0, 1))]})

            # ---- V projection round r (dout [1024r, 1024r+1024)), split into
            # bias-init / per-kt-pair matmuls / evac so attention mini-phases
            # can braid between chunks ----
            def v_bias(r):
                pv = [
                    [pvps.tile([128, 512], F32, name="pv", tag="pv") for _ in range(2)]
                    for _ in range(2)
                ]
                for tt in range(2):
                    for di in range(2):
                        sl = slice(r * 1024 + di * 512, r * 1024 + di * 512 + 512)
                        nc.tensor.matmul(
                            pv[tt][di][:], lhsT=ones1[:], rhs=bv_sb[:, sl],
                            start=True, stop=False,
                        )
                return pv

            def v_kp(r, pv, kp, vch=None):
                if vch is None:
                    vch = vch_load(r, kp)
                for half in range(2):
                    kt = 2 * kp + half
                    last = kt == NK - 1
                    for tt in range(2):
                        for di in range(2):
                            nc.tensor.matmul(
                                pv[tt][di][:],
                                lhsT=xv_sb[:, kt, tt * 128 : (tt + 1) * 128],
                                rhs=vch[:, half * 1024 + di * 512 : half * 1024 + (di + 1) * 512],
                                start=False,
                                stop=last,
                            )

            def v_evac(r, pv):
                for tt in range(2):
                    vdst = v0_sb if tt == 0 else v1_sb
                    for di in range(2):
                        sl = slice(r * 1024 + di * 512, r * 1024 + di * 512 + 512)
                        nc.scalar.activation(
                            out=vdst[:, sl], in_=pv[tt][di][:], func=Ident
                        )

            # ---- attention mini-phases ----
            hsl = [_head_slices(h, nch) for h in range(H)]

            def scores(h):
                pss = paps.tile([128, 512], F32, name="pa", tag="pa")
                for tt in range(2):
                    dst = pss[:, tt * 256 : tt * 256 + 256]
                    tsl = slice(tt * 128, (tt + 1) * 128)
                    for i, (t, o, e) in enumerate(hsl[h]):
                        nc.tensor.matmul(
                            dst,
                            lhsT=kt_sb[o:e, t, tsl],
                            rhs=qt_sb[o:e, t, :],
                            start=(i == 0),
                            stop=(i == len(hsl[h]) - 1),
                        )
                nc.scalar.activation(
                    out=et_sb[:, h, :, :], in_=pss[:], func=Exp
                )

            def colsum_recip(h):
                prs = paps.tile([128, 512], F32, name="pr", tag="pa")
                nc.tensor.matmul(
                    prs[0:1, 0:256], lhsT=ones128[:], rhs=et_sb[:, h, 0, :],
                    start=True, stop=False,
                )
                nc.tensor.matmul(
                    prs[0:1, 0:256], lhsT=ones128[:], rhs=et_sb[:, h, 1, :],
                    start=False, stop=True,
                )
                nc.vector.reciprocal(rinv[0:1, h, :], prs[0:1, 0:256])

            def bcast(p):
                pb = paps.tile([128, 512], F32, name="pb", tag="pa")
                nc.tensor.matmul(
                    pb[:],
                    lhsT=ones_f[:] if use_f32_bcast else ones1[:],
                    rhs=rinv[0:1, 2 * p : 2 * p + 2, :],
                    start=True, stop=True,
                )
                nc.scalar.activation(
                    out=rbc[:, 2 * p : 2 * p + 2, :], in_=pb[:], func=Ident
                )

            def phase_b(h):
                pu = paps.tile([128, 512], F32, name="pc", tag="pa")
                for j in range(2):
                    csl = slice(h * HD + j * 128, h * HD + (j + 1) * 128)
                    dst = pu[:, j * 256 : j * 256 + 256]
                    nc.tensor.matmul(
                        dst, lhsT=v0_sb[:, csl], rhs=et_sb[:, h, 0, :],
                        start=True, stop=False,
                    )
                    nc.tensor.matmul(
                        dst, lhsT=v1_sb[:, csl], rhs=et_sb[:, h, 1, :],
                        start=False, stop=True,
                    )
                for j in range(2):
                    nc.vector.tensor_mul(
                        at_sb[:, 2 * h + j, :],
                        pu[:, j * 256 : j * 256 + 256],
                        rbc[:, h, :],
                    )

            # ---- the braid ----
            pv = v_bias(0)
            for kp in range(16):
                v_kp(0, pv, kp, vch=warm_v0[kp] if kp < len(warm_v0) else None)
            v_evac(0, pv)

            for h in range(4):
                scores(h)          # fills round-0 evac latency
            pv = v_bias(1)
            for g in range(3):
                for h in range(4 * g + 4, 4 * g + 8):
                    scores(h)
                for h in range(4 * g, 4 * g + 4):
                    colsum_recip(h)
                for kp in range(5 * g, 5 * g + 5):
                    v_kp(1, pv, kp)
            v_kp(1, pv, 15)
            v_evac(1, pv)

            pv = v_bias(2)
            colsum_recip(12)
            colsum_recip(13)
            v_kp(2, pv, 0)
            v_kp(2, pv, 1)
            colsum_recip(14)
            colsum_recip(15)
            v_kp(2, pv, 2)
            bcast(0)
            bcast(1)
            for kp in range(3, 8):
                v_kp(2, pv, kp)
            bcast(2)
            bcast(3)
            for kp in range(8, 12):
                v_kp(2, pv, kp)
            for p in range(4, 8):
                bcast(p)
            for kp in range(12, 16):
                v_kp(2, pv, kp)
            v_evac(2, pv)

            pv = v_bias(3)
            for g in range(3):
                for h in range(4 * g, 4 * g + 4):
                    phase_b(h)
                for kp in range(5 * g, 5 * g + 5):
                    v_kp(3, pv, kp)
            v_kp(3, pv, 15)
            v_evac(3, pv)
            for h in range(12, 16):
                phase_b(h)

            # ---- output projection ----
            proj_t(wo, at_sb, NC, None, bo_sb, store=out)

    return nc


_cached = {}


def _get_nc(nch=None):
    if nch is None:
        nch = _cached.get("nch", 192)
    key = ("nc", nch)
    if key not in _cached:
        _cached[key] = build_nc(nch)
        _cached["nch"] = nch
    return _cached[key]


def _pick_nch(mask):
    mx = int((np.asarray(mask) != 0).sum(axis=1).max())
    for cand in (128, 192, 256):
        if mx <= cand:
            return cand
    return 256


def _proj_slab(Wt, n_dt):
    """W.T (k-major [D, dout]) -> [dout tile, 128 k-in-tile, NK kt, 128 dout]."""
    d_out = n_dt * 128
    a = Wt.reshape(NK, 128, n_dt, 128)  # [kt, p, dt, c]
    return np.ascontiguousarray(a.transpose(2, 1, 0, 3)).astype(NPBF16)


def _prep_shared(Wv, Wo, bo):
    wt = Wv.T.astype(NPBF16)  # [k, dout]
    # [round, kt-pair, 128 k, (2 half x 1024 dout)]
    wv_c = np.ascontiguousarray(
        wt.reshape(NK // 2, 2, 128, 4, 1024).transpose(3, 0, 2, 1, 4).reshape(
            4, NK // 2, 128, 2048
        )
    )
    wo_c = _proj_slab((Wo.T / 64.0).astype(np.float32), NC)
    bov = np.ascontiguousarray(bo.astype(np.float32).reshape(NC, 128).T)
    return wv_c, wo_c, bov


def build_in_maps(q, k, v, mask, Wq, bq, Wk, bk, Wv, bv, Wo, bo):
    q = np.asarray(q, dtype=np.float32)
    k = np.asarray(k, dtype=np.float32)
    v = np.asarray(v, dtype=np.float32)
    mask = np.asarray(mask)
    Wq, bq = np.asarray(Wq, np.float32), np.asarray(bq, np.float32)
    Wk, bk = np.asarray(Wk, np.float32), np.asarray(bk, np.float32)
    Wv, bv = np.asarray(Wv, np.float32), np.asarray(bv, np.float32)
    Wo, bo = np.asarray(Wo, np.float32), np.asarray(bo, np.float32)

    nch = _pick_nch(mask)
    _cached["nch"] = nch
    nqt = nch * H // 128

    wv_c, wo_c, bov = _prep_shared(Wv, Wo, bo)
    bvv = np.ascontiguousarray(bv.astype(NPBF16).reshape(1, D))

    in_maps = []
    for b in range(B):
        surv = np.nonzero(mask[b] != 0)[0]  # surviving hd slots
        n_b = len(surv)
        # packed channel index list per head, zero-padded to nch
        idx = np.zeros((H, nch), dtype=np.int64)
        valid = np.zeros((H, nch), dtype=bool)
        for h in range(H):
            idx[h, :n_b] = h * HD + surv
            valid[h, :n_b] = True
        idx = idx.reshape(-1)
        valid = valid.reshape(-1)

        def pack_w(W, scale):
            wt = W.T * scale  # [k, dout]
            g = wt[:, idx].astype(np.float32)
            g[:, ~valid] = 0.0
            return _proj_slab(g, nqt)

        def pack_b(bias, scale):
            g = (bias[idx] * scale).astype(np.float32)
            g[~valid] = 0.0
            return np.ascontiguousarray(g.reshape(nqt, 128).T)

        wq_c = pack_w(Wq, 1.0 / 16.0)
        wk_c = pack_w(Wk, 1.0)
        bqv = pack_b(bq, 1.0 / 16.0)
        bkv = pack_b(bk, 1.0)

        def xt(x):
            t = x[b].T.reshape(NK, 128, S).swapaxes(0, 1)
            return np.ascontiguousarray(t).astype(NPBF16)

        in_maps.append(
            dict(
                xq=xt(q), xk=xt(k), xv=xt(v),
                wq=wq_c, wk=wk_c, wv=wv_c, wo=wo_c,
                bqv=bqv, bkv=bkv, bov=bov, bvv=bvv,
            )
        )
    return in_maps


def unshard(results):
    outs = []
    for b in range(B):
        ot = results[b]["out"]  # [32, 128, 256]
        outs.append(ot.reshape(D, S).T)  # [256, 4096]
    return np.ascontiguousarray(np.stack(outs)).astype(np.float32)


def kernel(q, k, v, mask, Wq, bq, Wk, bk, Wv, bv, Wo, bo):
    _install_ntff_hook()
    _patch_bir_wait_split()
    in_maps = build_in_maps(q, k, v, mask, Wq, bq, Wk, bk, Wv, bv, Wo, bo)
    nc = _get_nc()

    from concourse.bass_utils import run_bass_kernel_spmd

    res = run_bass_kernel_spmd(nc, in_maps, list(range(N_CORES)))
    return unshard(res.results)
